# revision 3
# baseline (speedup 1.0000x reference)
"""LoftQ linear layer on 8 TRN2 NeuronCores.

Computes out = x @ (W_q + 1/16 * A @ B).T + bias for
x:(4,4096,4096) W_q:(4096,4096) A:(4096,16) B:(16,4096) bias:(4096,).

Sharding: data-parallel over the 16384 flattened rows of x (2048 rows per
core); W_q/A/B/bias replicated. Each core:
  1. eff_w prep: AB tile = A.T-matmul with pre-scaled B on PE (K=16),
     evicted as eff_w = psum + W_q with an f32->bf16 cast, round-tripped
     through DRAM so the 2-byte HW DMA-transpose can serve eff_w.T tiles.
  2. x prep: f32->bf16 cast round-trip, DMA-transposed into SBUF-resident
     x.T tiles, one 1024-row half at a time.
  3. main loop: psum[m=128,o=512] += xT[i]. T @ wT[i,o] over 32 K-tiles,
     bias (broadcast across partitions once via a K=1 matmul) added during
     PSUM eviction.
"""

import numpy as np

P = 128
M_CORE = 2048  # rows of x per core
K = 4096  # D_in (contraction)
N = 4096  # D_out
R = 16  # LoRA rank
SCALING = 1.0 / 16.0
N_CORES = 8

M_HALF = 1024  # x.T residency granularity
OB = 512  # output-feature block (psum free dim)
N_OB = N // OB  # 8
N_IT = K // P  # 32 contraction tiles
N_MS = M_HALF // P  # 8 m-subtiles per half

_CACHE = {}
LAST_RESULT = None


def _build_nc():
    from contextlib import ExitStack

    from concourse import bacc, mybir, tile
    from concourse.masks import make_identity

    f32 = mybir.dt.float32
    bf16 = mybir.dt.bfloat16
    add = mybir.AluOpType.add

    nc = bacc.Bacc(None, target_bir_lowering=False, debug=False)

    x = nc.declare_dram_parameter("x", [M_CORE, K], f32, isOutput=False)
    wq = nc.declare_dram_parameter("W_q", [N, K], f32, isOutput=False)
    a = nc.declare_dram_parameter("A", [N, R], f32, isOutput=False)
    b = nc.declare_dram_parameter("B", [R, K], f32, isOutput=False)
    bias = nc.declare_dram_parameter("bias", [1, N], f32, isOutput=False)
    out = nc.declare_dram_parameter("out", [M_CORE, N], f32, isOutput=True)

    xbf = nc.dram_tensor("xbf", [M_CORE, K], bf16)
    wbf = nc.dram_tensor("wbf", [N, K], bf16)

    with tile.TileContext(nc) as tc, ExitStack() as ctx:
        const = ctx.enter_context(tc.tile_pool(name="const", bufs=1))
        stage = ctx.enter_context(tc.tile_pool(name="stage", bufs=2))
        cast_p = ctx.enter_context(tc.tile_pool(name="cast", bufs=2))
        wbf_p = ctx.enter_context(tc.tile_pool(name="wbf_p", bufs=4))
        wt_p = ctx.enter_context(tc.tile_pool(name="wt_p", bufs=64))
        xt_p = ctx.enter_context(tc.tile_pool(name="xt_p", bufs=288))
        out_p = ctx.enter_context(tc.tile_pool(name="out_p", bufs=4))
        bias_p = ctx.enter_context(tc.tile_pool(name="bias_p", bufs=2))
        ps_main = ctx.enter_context(tc.tile_pool(name="ps_main", bufs=4, space="PSUM"))
        ps_ab = ctx.enter_context(tc.tile_pool(name="ps_ab", bufs=2, space="PSUM"))
        ps_misc = ctx.enter_context(tc.tile_pool(name="ps_misc", bufs=1, space="PSUM"))

        # ---- constants / small operands --------------------------------
        identity = const.tile([P, P], bf16, tag="identity")
        make_identity(nc, identity)
        ones = const.tile([1, P], f32, tag="ones")
        nc.vector.memset(ones[:], 1.0)

        # B scaled by 1/16 and cast to bf16: B_bf[r, i]
        b_bf = const.tile([R, K], bf16, tag="b_bf")
        for c in range(4):
            t = stage.tile([P, 1024], f32, tag="stage")
            nc.sync.dma_start(out=t[:R, :], in_=b[:, c * 1024 : (c + 1) * 1024])
            nc.vector.tensor_scalar_mul(b_bf[:, c * 1024 : (c + 1) * 1024], t[:R, :], SCALING)

        # A transposed to AT_bf[r, o] via PE transpose
        at_bf = const.tile([R, N], bf16, tag="at_bf")
        for j in range(N // P):
            t = stage.tile([P, 1024], f32, tag="stage")
            nc.sync.dma_start(out=t[:, :R], in_=a[j * P : (j + 1) * P, :])
            tb = cast_p.tile([P, R], bf16, tag="a_bf")
            nc.vector.tensor_copy(tb[:], t[:, :R])
            ps = ps_misc.tile([R, P], bf16, tag="ps_at")
            nc.tensor.transpose(ps[:], tb[:], identity[:])
            nc.vector.tensor_copy(at_bf[:, j * P : (j + 1) * P], ps[:])

        # ---- W prep: wbf = bf16(W_q + A @ (B/16)) ----------------------
        for os_ in range(N // P):
            for c in range(4):
                wq_s = stage.tile([P, 1024], f32, tag="wq_s")
                nc.sync.dma_start(
                    out=wq_s[:], in_=wq[os_ * P : (os_ + 1) * P, c * 1024 : (c + 1) * 1024]
                )
                for h in range(2):
                    ic = 2 * c + h
                    ps = ps_ab.tile([P, OB], f32, tag="ps_ab")
                    nc.tensor.matmul(
                        ps[:],
                        lhsT=at_bf[:, os_ * P : (os_ + 1) * P],
                        rhs=b_bf[:, ic * OB : (ic + 1) * OB],
                        start=True,
                        stop=True,
                    )
                    wt = wbf_p.tile([P, OB], bf16, tag="wbf_t")
                    nc.vector.tensor_tensor(wt[:], ps[:], wq_s[:, h * OB : (h + 1) * OB], add)
                    nc.sync.dma_start(
                        out=wbf[os_ * P : (os_ + 1) * P, ic * OB : (ic + 1) * OB], in_=wt[:]
                    )

        # ---- per half: x prep + main matmul ----------------------------
        for half in range(2):
            # x prep: cast rows to bf16, round-trip, transpose-load resident xT
            xt = {}
            for ms in range(N_MS):
                row0 = half * M_HALF + ms * P
                for c in range(4):
                    xs = stage.tile([P, 1024], f32, tag="x_s")
                    nc.sync.dma_start(out=xs[:], in_=x[row0 : row0 + P, c * 1024 : (c + 1) * 1024])
                    xc = cast_p.tile([P, 1024], bf16, tag="x_bf")
                    nc.vector.tensor_copy(xc[:], xs[:])
                    nc.sync.dma_start(
                        out=xbf[row0 : row0 + P, c * 1024 : (c + 1) * 1024], in_=xc[:]
                    )
                for it in range(N_IT):
                    t = xt_p.tile([P, P], bf16, tag="xT")
                    nc.sync.dma_start(
                        out=t[:],
                        in_=xbf[row0 : row0 + P, it * P : (it + 1) * P],
                        transpose=True,
                    )
                    xt[(it, ms)] = t

            # main loop over output blocks
            for ob in range(N_OB):
                o0 = ob * OB
                # bias broadcast across partitions via K=1 matmul
                bs = stage.tile([P, 1024], f32, tag="bias_s")
                nc.sync.dma_start(out=bs[:1, :OB], in_=bias[:, o0 : o0 + OB])
                ps_b = ps_misc.tile([P, OB], f32, tag="ps_bias")
                nc.tensor.matmul(ps_b[:], lhsT=ones[:], rhs=bs[:1, :OB], start=True, stop=True)
                bias_bc = bias_p.tile([P, OB], f32, tag="bias_bc")
                nc.vector.tensor_copy(bias_bc[:], ps_b[:])

                wt_tiles = []
                for it in range(N_IT):
                    t = wt_p.tile([P, OB], bf16, tag="wT")
                    nc.sync.dma_start(
                        out=t[:],
                        in_=wbf[o0 : o0 + OB, it * P : (it + 1) * P],
                        transpose=True,
                    )
                    wt_tiles.append(t)

                for ms in range(N_MS):
                    ps = ps_main.tile([P, OB], f32, tag="ps_main")
                    for it in range(N_IT):
                        nc.tensor.matmul(
                            ps[:],
                            lhsT=xt[(it, ms)][:],
                            rhs=wt_tiles[it][:],
                            start=(it == 0),
                            stop=(it == N_IT - 1),
                        )
                    o_sb = out_p.tile([P, OB], f32, tag="o_sb")
                    nc.vector.tensor_tensor(o_sb[:], ps[:], bias_bc[:], add)
                    row0 = half * M_HALF + ms * P
                    nc.sync.dma_start(out=out[row0 : row0 + P, o0 : o0 + OB], in_=o_sb[:])

    nc.compile()
    return nc


def get_nc():
    if "nc" not in _CACHE:
        _CACHE["nc"] = _build_nc()
    return _CACHE["nc"]


def kernel(**inputs):
    global LAST_RESULT
    from concourse.bass_utils import run_bass_kernel_spmd

    x = np.ascontiguousarray(np.asarray(inputs["x"], dtype=np.float32)).reshape(-1, K)
    wq = np.ascontiguousarray(np.asarray(inputs["W_q"], dtype=np.float32))
    a = np.ascontiguousarray(np.asarray(inputs["A"], dtype=np.float32))
    b = np.ascontiguousarray(np.asarray(inputs["B"], dtype=np.float32))
    bias = np.ascontiguousarray(np.asarray(inputs["bias"], dtype=np.float32)).reshape(1, N)

    nc = get_nc()
    in_maps = [
        {
            "x": x[c * M_CORE : (c + 1) * M_CORE],
            "W_q": wq,
            "A": a,
            "B": b,
            "bias": bias,
        }
        for c in range(N_CORES)
    ]
    res = run_bass_kernel_spmd(nc, in_maps, core_ids=list(range(N_CORES)))
    LAST_RESULT = res
    out = np.concatenate([res.results[c]["out"] for c in range(N_CORES)], axis=0)
    return out.reshape(4, 4096, 4096)


# revision 6
# speedup vs baseline: 1.3320x; 1.3320x over previous
"""LoftQ linear layer on 8 TRN2 NeuronCores.

Computes out = x @ (W_q + 1/16 * A @ B).T + bias for
x:(4,4096,4096) W_q:(4096,4096) A:(4096,16) B:(16,4096) bias:(4096,).

Sharding: data-parallel over the 16384 flattened rows of x (2048 rows per
core); W_q/A/B/bias replicated. Per core:
  1. W prep: AB psum tile (PE, K=16, pre-scaled B) + W_q added during the
     f32->bf16 eviction; the bf16 eff_w tile is PE-transposed and stored to
     DRAM in transposed layout wbfT[K, N], so main-loop weight loads are
     natural batched 3D DMAs instead of per-tile DMA transposes.
  2. x prep: f32->bf16 cast, round-trip through DRAM, 2-byte HW DMA
     transpose into SBUF-resident x.T tiles ([128, 1024] grain, one
     1024-row half at a time).
  3. main loop: psum[m=128, o=512] += xT[i].T @ wT[i, o] over 32 K-tiles;
     bias (broadcast across partitions via a K=1 matmul) added during the
     PSUM eviction.
DMA issue is spread: natural loads/stores on nc.sync (HWDGE/SP), the few
remaining DMA transposes on nc.scalar (HWDGE/ACT), output + small stores
on nc.gpsimd (SWDGE); transpose-psum copybacks run on the idle ScalarE.
"""

import numpy as np

P = 128
M_CORE = 2048  # rows of x per core
K = 4096  # D_in (contraction)
N = 4096  # D_out
R = 16  # LoRA rank
SCALING = 1.0 / 16.0
N_CORES = 8

M_HALF = 1024  # x.T residency granularity
OB = 512  # output-feature block (psum free dim)
N_OB = N // OB  # 8
N_IT = K // P  # 32 contraction tiles
N_MS = M_HALF // P  # 8 m-subtiles per half

_CACHE = {}
LAST_RESULT = None


def _build_nc():
    from contextlib import ExitStack

    from concourse import bacc, mybir, tile
    from concourse.masks import make_identity

    f32 = mybir.dt.float32
    bf16 = mybir.dt.bfloat16
    add = mybir.AluOpType.add
    Copy = mybir.ActivationFunctionType.Copy

    nc = bacc.Bacc(None, target_bir_lowering=False, debug=False)

    x = nc.declare_dram_parameter("x", [M_CORE, K], f32, isOutput=False)
    wq = nc.declare_dram_parameter("W_q", [N, K], f32, isOutput=False)
    a = nc.declare_dram_parameter("A", [N, R], f32, isOutput=False)
    b = nc.declare_dram_parameter("B", [R, K], f32, isOutput=False)
    bias = nc.declare_dram_parameter("bias", [1, N], f32, isOutput=False)
    out = nc.declare_dram_parameter("out", [M_CORE, N], f32, isOutput=True)

    xbf = nc.dram_tensor("xbf", [M_CORE, K], bf16)
    wbfT = nc.dram_tensor("wbfT", [K, N], bf16)  # eff_w.T, bf16

    with tile.TileContext(nc) as tc, ExitStack() as ctx:
        const = ctx.enter_context(tc.tile_pool(name="const", bufs=1))
        stage = ctx.enter_context(tc.tile_pool(name="stage", bufs=2))
        weff_p = ctx.enter_context(tc.tile_pool(name="weff_p", bufs=8))
        wbfT_p = ctx.enter_context(tc.tile_pool(name="wbfT_p", bufs=4))
        wt_p = ctx.enter_context(tc.tile_pool(name="wt_p", bufs=12))
        xt_p = ctx.enter_context(tc.tile_pool(name="xt_p", bufs=40))
        out_p = ctx.enter_context(tc.tile_pool(name="out_p", bufs=4))
        bias_p = ctx.enter_context(tc.tile_pool(name="bias_p", bufs=2))
        ps_main = ctx.enter_context(tc.tile_pool(name="ps_main", bufs=4, space="PSUM"))
        ps_ab = ctx.enter_context(tc.tile_pool(name="ps_ab", bufs=2, space="PSUM"))
        ps_t = ctx.enter_context(tc.tile_pool(name="ps_t", bufs=2, space="PSUM"))

        # ---- constants / small operands --------------------------------
        identity = const.tile([P, P], bf16, tag="identity")
        make_identity(nc, identity)
        ones = const.tile([1, P], f32, tag="ones")
        nc.vector.memset(ones[:], 1.0)

        # B scaled by 1/16, cast to bf16: b_bf[r, i]
        b_bf = const.tile([R, K], bf16, tag="b_bf")
        for c in range(2):
            t = stage.tile([P, 2048], f32, tag="x_s")
            nc.sync.dma_start(out=t[:R, :], in_=b[:, c * 2048 : (c + 1) * 2048])
            nc.vector.tensor_scalar_mul(b_bf[:, c * 2048 : (c + 1) * 2048], t[:R, :], SCALING)

        # A transposed to at_bf[r, o] via PE transpose
        at_bf = const.tile([R, N], bf16, tag="at_bf")
        for j in range(N // P):
            t = stage.tile([P, 2048], f32, tag="x_s")
            nc.sync.dma_start(out=t[:, :R], in_=a[j * P : (j + 1) * P, :])
            tb = stage.tile([P, R], bf16, tag="a_bf")
            nc.vector.tensor_copy(tb[:], t[:, :R])
            ps = ps_t.tile([R, P], bf16, tag="ps_tr")
            nc.tensor.transpose(ps[:], tb[:], identity[:])
            nc.vector.tensor_copy(at_bf[:, j * P : (j + 1) * P], ps[:])

        # ---- W prep: wbfT = bf16(W_q + A @ (B/16)).T -------------------
        for ob in range(N_OB):
            for ic in range(8):  # 512-wide i-chunks
                i0 = ic * 512
                weff = []
                for os_ in range(4):
                    o_row = (ob * 4 + os_) * P
                    wq_s = stage.tile([P, 512], f32, tag="wq_s")
                    nc.sync.dma_start(out=wq_s[:], in_=wq[o_row : o_row + P, i0 : i0 + 512])
                    ps = ps_ab.tile([P, 512], f32, tag="ps_ab")
                    nc.tensor.matmul(
                        ps[:],
                        lhsT=at_bf[:, o_row : o_row + P],
                        rhs=b_bf[:, i0 : i0 + 512],
                        start=True,
                        stop=True,
                    )
                    we = weff_p.tile([P, 512], bf16, tag="weff")
                    nc.vector.tensor_tensor(we[:], ps[:], wq_s[:], add)
                    weff.append(we)
                # transpose the [o=512, i=512] block into wbfT rows
                for it2 in range(4):
                    i_row = i0 + it2 * P
                    wtt = wbfT_p.tile([P, 512], bf16, tag="wbfT_t")
                    for os_ in range(4):
                        pst = ps_t.tile([P, P], bf16, tag="ps_tr")
                        nc.tensor.transpose(
                            pst[:], weff[os_][:, it2 * P : (it2 + 1) * P], identity[:]
                        )
                        nc.scalar.activation(wtt[:, os_ * P : (os_ + 1) * P], pst[:], Copy)
                    nc.sync.dma_start(
                        out=wbfT[i_row : i_row + P, ob * OB : (ob + 1) * OB], in_=wtt[:]
                    )

        # ---- per half: x prep + main matmul ----------------------------
        for half in range(2):
            # x prep: cast rows to bf16, round-trip, batched transpose-load
            for ms in range(N_MS):
                row0 = half * M_HALF + ms * P
                for c in range(2):
                    xs = stage.tile([P, 2048], f32, tag="x_s")
                    nc.sync.dma_start(
                        out=xs[:], in_=x[row0 : row0 + P, c * 2048 : (c + 1) * 2048]
                    )
                    xc = stage.tile([P, 2048], bf16, tag="x_bf")
                    nc.vector.tensor_copy(xc[:], xs[:])
                    nc.sync.dma_start(
                        out=xbf[row0 : row0 + P, c * 2048 : (c + 1) * 2048], in_=xc[:]
                    )
            xt = []
            for it in range(N_IT):
                t = xt_p.tile([P, M_HALF], bf16, tag="xT")
                nc.scalar.dma_start(
                    out=t[:],
                    in_=xbf[half * M_HALF : (half + 1) * M_HALF, it * P : (it + 1) * P],
                    transpose=True,
                )
                xt.append(t)

            # main loop over output blocks
            for ob in range(N_OB):
                o0 = ob * OB
                # bias broadcast across partitions via K=1 matmul
                bs = stage.tile([1, OB], f32, tag="bias_s")
                nc.gpsimd.dma_start(out=bs[:], in_=bias[:, o0 : o0 + OB])
                ps_b = ps_ab.tile([P, OB], f32, tag="ps_ab")
                nc.tensor.matmul(ps_b[:], lhsT=ones[:], rhs=bs[:], start=True, stop=True)
                bias_bc = bias_p.tile([P, OB], f32, tag="bias_bc")
                nc.vector.tensor_copy(bias_bc[:], ps_b[:])

                # batched natural loads of eff_w.T: [128, 4, 512] per DMA
                wt_tiles = []
                for g in range(8):
                    t = wt_p.tile([P, 4, OB], bf16, tag="wT")
                    src = wbfT[g * 512 : (g + 1) * 512, o0 : o0 + OB]
                    nc.sync.dma_start(out=t[:], in_=src.rearrange("(k p) n -> p k n", p=P))
                    wt_tiles.append(t)

                for ms in range(N_MS):
                    ps = ps_main.tile([P, OB], f32, tag="ps_main")
                    for it in range(N_IT):
                        nc.tensor.matmul(
                            ps[:],
                            lhsT=xt[it][:, ms * P : (ms + 1) * P],
                            rhs=wt_tiles[it // 4][:, it % 4, :],
                            start=(it == 0),
                            stop=(it == N_IT - 1),
                        )
                    o_sb = out_p.tile([P, OB], f32, tag="o_sb")
                    nc.vector.tensor_tensor(o_sb[:], ps[:], bias_bc[:], add)
                    row0 = half * M_HALF + ms * P
                    nc.gpsimd.dma_start(out=out[row0 : row0 + P, o0 : o0 + OB], in_=o_sb[:])

    nc.compile()
    return nc


def get_nc():
    if "nc" not in _CACHE:
        _CACHE["nc"] = _build_nc()
    return _CACHE["nc"]


def kernel(**inputs):
    global LAST_RESULT
    from concourse.bass_utils import run_bass_kernel_spmd

    x = np.ascontiguousarray(np.asarray(inputs["x"], dtype=np.float32)).reshape(-1, K)
    wq = np.ascontiguousarray(np.asarray(inputs["W_q"], dtype=np.float32))
    a = np.ascontiguousarray(np.asarray(inputs["A"], dtype=np.float32))
    b = np.ascontiguousarray(np.asarray(inputs["B"], dtype=np.float32))
    bias = np.ascontiguousarray(np.asarray(inputs["bias"], dtype=np.float32)).reshape(1, N)

    nc = get_nc()
    in_maps = [
        {
            "x": x[c * M_CORE : (c + 1) * M_CORE],
            "W_q": wq,
            "A": a,
            "B": b,
            "bias": bias,
        }
        for c in range(N_CORES)
    ]
    res = run_bass_kernel_spmd(nc, in_maps, core_ids=list(range(N_CORES)))
    LAST_RESULT = res
    out = np.concatenate([res.results[c]["out"] for c in range(N_CORES)], axis=0)
    return out.reshape(4, 4096, 4096)


# revision 8
# speedup vs baseline: 1.4835x; 1.1138x over previous
"""LoftQ linear layer on 8 TRN2 NeuronCores.

Computes out = x @ (W_q + 1/16 * A @ B).T + bias for
x:(4,4096,4096) W_q:(4096,4096) A:(4096,16) B:(16,4096) bias:(4096,).

Sharding: data-parallel over the 16384 flattened rows of x (2048 rows per
core); W_q/A/B/bias replicated. Per core:
  1. W prep: AB psum tile (PE, K=16, pre-scaled B) + W_q added during the
     f32->bf16 eviction; the bf16 eff_w tile is PE-transposed and stored to
     DRAM in transposed layout wbfT[K, N], so main-loop weight loads are
     natural batched 3D DMAs instead of per-tile DMA transposes.
  2. x prep: f32->bf16 cast, round-trip through DRAM, 2-byte HW DMA
     transpose into SBUF-resident x.T tiles ([128, 1024] grain, one
     1024-row half at a time).
  3. main loop: psum[m=128, o=512] += xT[i].T @ wT[i, o] over 32 K-tiles;
     bias (broadcast across partitions via a K=1 matmul) added during the
     PSUM eviction.
DMA issue is spread: natural loads/stores on nc.sync (HWDGE/SP), the few
remaining DMA transposes on nc.scalar (HWDGE/ACT), output + small stores
on nc.gpsimd (SWDGE); transpose-psum copybacks run on the idle ScalarE.
"""

import numpy as np

P = 128
M_CORE = 2048  # rows of x per core
K = 4096  # D_in (contraction)
N = 4096  # D_out
R = 16  # LoRA rank
SCALING = 1.0 / 16.0
N_CORES = 8

M_HALF = 1024  # x.T residency granularity
OB = 512  # output-feature block (psum free dim)
N_OB = N // OB  # 8
N_IT = K // P  # 32 contraction tiles
N_MS = M_HALF // P  # 8 m-subtiles per half

_CACHE = {}
LAST_RESULT = None


def _build_nc():
    from contextlib import ExitStack

    from concourse import bacc, mybir, tile
    from concourse.masks import make_identity

    f32 = mybir.dt.float32
    bf16 = mybir.dt.bfloat16
    add = mybir.AluOpType.add
    Copy = mybir.ActivationFunctionType.Copy

    nc = bacc.Bacc(None, target_bir_lowering=False, debug=False)

    x = nc.declare_dram_parameter("x", [M_CORE, K], f32, isOutput=False)
    wq = nc.declare_dram_parameter("W_q", [N, K], f32, isOutput=False)
    a = nc.declare_dram_parameter("A", [N, R], f32, isOutput=False)
    b = nc.declare_dram_parameter("B", [R, K], f32, isOutput=False)
    bias = nc.declare_dram_parameter("bias", [1, N], f32, isOutput=False)
    out = nc.declare_dram_parameter("out", [M_CORE, N], f32, isOutput=True)

    xbf = nc.dram_tensor("xbf", [M_CORE, K], bf16)
    wbfT = nc.dram_tensor("wbfT", [K, N], bf16)  # eff_w.T, bf16

    with tile.TileContext(nc) as tc, ExitStack() as ctx:
        const = ctx.enter_context(tc.tile_pool(name="const", bufs=1))
        stage = ctx.enter_context(tc.tile_pool(name="stage", bufs=2))
        weff_p = ctx.enter_context(tc.tile_pool(name="weff_p", bufs=8))
        wbfT_p = ctx.enter_context(tc.tile_pool(name="wbfT_p", bufs=4))
        wt_p = ctx.enter_context(tc.tile_pool(name="wt_p", bufs=12))
        xt_p = ctx.enter_context(tc.tile_pool(name="xt_p", bufs=40))
        out_p = ctx.enter_context(tc.tile_pool(name="out_p", bufs=4))
        bias_p = ctx.enter_context(tc.tile_pool(name="bias_p", bufs=2))
        ps_main = ctx.enter_context(tc.tile_pool(name="ps_main", bufs=2, space="PSUM"))
        ps_ab = ctx.enter_context(tc.tile_pool(name="ps_ab", bufs=2, space="PSUM"))
        ps_t = ctx.enter_context(tc.tile_pool(name="ps_t", bufs=4, space="PSUM"))

        # ---- constants / small operands --------------------------------
        identity = const.tile([P, P], bf16, tag="identity")
        make_identity(nc, identity)
        ones = const.tile([1, P], f32, tag="ones")
        nc.vector.memset(ones[:], 1.0)

        # B scaled by 1/16, cast to bf16: b_bf[r, i]
        b_bf = const.tile([R, K], bf16, tag="b_bf")
        for c in range(2):
            t = stage.tile([P, 2048], f32, tag="x_s")
            nc.sync.dma_start(out=t[:R, :], in_=b[:, c * 2048 : (c + 1) * 2048])
            nc.vector.tensor_scalar_mul(b_bf[:, c * 2048 : (c + 1) * 2048], t[:R, :], SCALING)

        # A transposed to at_bf[r, o] via PE transpose
        at_bf = const.tile([R, N], bf16, tag="at_bf")
        for j in range(N // P):
            t = stage.tile([P, 2048], f32, tag="x_s")
            nc.sync.dma_start(out=t[:, :R], in_=a[j * P : (j + 1) * P, :])
            tb = stage.tile([P, R], bf16, tag="a_bf")
            nc.vector.tensor_copy(tb[:], t[:, :R])
            ps = ps_t.tile([R, P], bf16, tag="ps_tr")
            nc.tensor.transpose(ps[:], tb[:], identity[:])
            nc.vector.tensor_copy(at_bf[:, j * P : (j + 1) * P], ps[:])

        # ---- W prep: wbfT = bf16(W_q + A @ (B/16)).T -------------------
        def w_prep(ob):
            for ic in range(8):  # 512-wide i-chunks
                i0 = ic * 512
                weff = []
                for os_ in range(4):
                    o_row = (ob * 4 + os_) * P
                    wq_s = stage.tile([P, 512], f32, tag="wq_s")
                    nc.sync.dma_start(out=wq_s[:], in_=wq[o_row : o_row + P, i0 : i0 + 512])
                    ps = ps_ab.tile([P, 512], f32, tag="ps_ab")
                    nc.tensor.matmul(
                        ps[:],
                        lhsT=at_bf[:, o_row : o_row + P],
                        rhs=b_bf[:, i0 : i0 + 512],
                        start=True,
                        stop=True,
                    )
                    we = weff_p.tile([P, 512], bf16, tag="weff")
                    nc.vector.tensor_tensor(we[:], ps[:], wq_s[:], add)
                    weff.append(we)
                # transpose the [o=512, i=512] block into wbfT rows
                for it2 in range(4):
                    i_row = i0 + it2 * P
                    wtt = wbfT_p.tile([P, 512], bf16, tag="wbfT_t")
                    for os_ in range(4):
                        pst = ps_t.tile([P, P], bf16, tag="ps_tr")
                        nc.tensor.transpose(
                            pst[:], weff[os_][:, it2 * P : (it2 + 1) * P], identity[:]
                        )
                        dst = wtt[:, os_ * P : (os_ + 1) * P]
                        if (it2 + os_) % 3 == 0:
                            nc.vector.tensor_copy(dst, pst[:])
                        else:
                            nc.scalar.activation(dst, pst[:], Copy)
                    nc.sync.dma_start(
                        out=wbfT[i_row : i_row + P, ob * OB : (ob + 1) * OB], in_=wtt[:]
                    )

        def x_prep(half):
            # cast rows to bf16, round-trip, batched transpose-load
            for ms in range(N_MS):
                row0 = half * M_HALF + ms * P
                for c in range(2):
                    xs = stage.tile([P, 2048], f32, tag="x_s")
                    nc.sync.dma_start(
                        out=xs[:], in_=x[row0 : row0 + P, c * 2048 : (c + 1) * 2048]
                    )
                    xc = stage.tile([P, 2048], bf16, tag="x_bf")
                    nc.vector.tensor_copy(xc[:], xs[:])
                    nc.sync.dma_start(
                        out=xbf[row0 : row0 + P, c * 2048 : (c + 1) * 2048], in_=xc[:]
                    )
            xt = []
            for it in range(N_IT):
                t = xt_p.tile([P, M_HALF], bf16, tag="xT")
                nc.scalar.dma_start(
                    out=t[:],
                    in_=xbf[half * M_HALF : (half + 1) * M_HALF, it * P : (it + 1) * P],
                    transpose=True,
                )
                xt.append(t)
            return xt

        def main_block(half, ob, xt):
            o0 = ob * OB
            # bias broadcast across partitions via K=1 matmul
            bs = stage.tile([1, OB], f32, tag="bias_s")
            nc.gpsimd.dma_start(out=bs[:], in_=bias[:, o0 : o0 + OB])
            ps_b = ps_ab.tile([P, OB], f32, tag="ps_ab")
            nc.tensor.matmul(ps_b[:], lhsT=ones[:], rhs=bs[:], start=True, stop=True)
            bias_bc = bias_p.tile([P, OB], f32, tag="bias_bc")
            nc.vector.tensor_copy(bias_bc[:], ps_b[:])

            # batched natural loads of eff_w.T: [128, 4, 512] per DMA
            wt_tiles = []
            for g in range(8):
                t = wt_p.tile([P, 4, OB], bf16, tag="wT")
                src = wbfT[g * 512 : (g + 1) * 512, o0 : o0 + OB]
                nc.sync.dma_start(out=t[:], in_=src.rearrange("(k p) n -> p k n", p=P))
                wt_tiles.append(t)

            for ms in range(N_MS):
                ps = ps_main.tile([P, OB], f32, tag="ps_main")
                for it in range(N_IT):
                    nc.tensor.matmul(
                        ps[:],
                        lhsT=xt[it][:, ms * P : (ms + 1) * P],
                        rhs=wt_tiles[it // 4][:, it % 4, :],
                        start=(it == 0),
                        stop=(it == N_IT - 1),
                    )
                o_sb = out_p.tile([P, OB], f32, tag="o_sb")
                nc.vector.tensor_tensor(o_sb[:], ps[:], bias_bc[:], add)
                row0 = half * M_HALF + ms * P
                nc.gpsimd.dma_start(out=out[row0 : row0 + P, o0 : o0 + OB], in_=o_sb[:])

        # Interleave: keep W-prep two blocks ahead of the main loop so its
        # PE transposes weave into the matmul stream instead of forming a
        # serial prologue (scheduler priority follows program order).
        w_prep(0)
        w_prep(1)
        xt = x_prep(0)
        for ob in range(N_OB):
            main_block(0, ob, xt)
            if ob + 2 < N_OB:
                w_prep(ob + 2)
        xt = x_prep(1)
        for ob in range(N_OB):
            main_block(1, ob, xt)

    nc.compile()
    return nc


def get_nc():
    if "nc" not in _CACHE:
        _CACHE["nc"] = _build_nc()
    return _CACHE["nc"]


def kernel(**inputs):
    global LAST_RESULT
    from concourse.bass_utils import run_bass_kernel_spmd

    x = np.ascontiguousarray(np.asarray(inputs["x"], dtype=np.float32)).reshape(-1, K)
    wq = np.ascontiguousarray(np.asarray(inputs["W_q"], dtype=np.float32))
    a = np.ascontiguousarray(np.asarray(inputs["A"], dtype=np.float32))
    b = np.ascontiguousarray(np.asarray(inputs["B"], dtype=np.float32))
    bias = np.ascontiguousarray(np.asarray(inputs["bias"], dtype=np.float32)).reshape(1, N)

    nc = get_nc()
    in_maps = [
        {
            "x": x[c * M_CORE : (c + 1) * M_CORE],
            "W_q": wq,
            "A": a,
            "B": b,
            "bias": bias,
        }
        for c in range(N_CORES)
    ]
    res = run_bass_kernel_spmd(nc, in_maps, core_ids=list(range(N_CORES)))
    LAST_RESULT = res
    out = np.concatenate([res.results[c]["out"] for c in range(N_CORES)], axis=0)
    return out.reshape(4, 4096, 4096)
